# revision 4
# baseline (speedup 1.0000x reference)
"""Trainium2 Bass kernel for nn_DiagTripleRCell.

Math (per (b, d) element, T steps):
    xv = W_x x + b ; xd = W_delta x + b_delta ; xg = W_gate x + b_gate
    h_{t+1} = (1-delta_t) h_t + delta_t tanh(xv_t + r_h h_t),
      delta_t = sigmoid(xd_t + r_delta h_t)
    out_t = h_{t+1} * silu(xg_t)

Strategy: batch-parallel over 8 cores (B=16 -> 2 per core). GEMMs run on
the PE in f32r (tf32) with outputs laid [d_out partitions, time free] so
the recurrence can use the DVE hardware affine scan (tensor_tensor_scan).
The nonlinear recurrence is solved parallel-in-time by Newton iteration:
an h-independent affine-scan initial guess, then NITER Newton passes,
each of which linearizes around the current trajectory and solves the
resulting affine recurrence exactly with one scan per (d-chunk, b) pair.
Iteration 1 runs in fp16 (accuracy only needs to reach ~3e-3 there),
the final iteration in fp32.
"""
import sys

sys.path.insert(0, "/opt/trn_rl_repo")

import numpy as np

import concourse.bass as bass
import concourse.tile as tile
from concourse import mybir
from concourse.bass_utils import run_bass_kernel_spmd
from concourse.vector_clock import ScopedClock

F32 = mybir.dt.float32
F32R = mybir.dt.float32r
F16 = mybir.dt.float16
AF = mybir.ActivationFunctionType
ALU = mybir.AluOpType

T, B, D = 2048, 16, 1024
NCORES = 8
BL = B // NCORES          # batches per core
EC = D // 128             # output-d chunks
KC = D // 128             # contraction chunks
TC = T // 512             # psum column chunks
NITER = 2                 # Newton iterations after the init scan

# ---------------------------------------------------------------------------
# walrus workaround: this container's walrus accepts at most ONE sem wait per
# instruction; split extras onto single-wait NOPs.
_uid = [0]


def _nop_like(inst, wait):
    _uid[0] += 1
    return mybir.InstNoOp(
        name=f"waitnop_{_uid[0]}", ins=[], outs=[], engine=inst.engine,
        sync_info=mybir.SyncInfo(on_wait=[wait], on_update=[]),
    )


def _split_multi_waits(nc):
    for fn in nc.m.functions:
        for blk in fn.blocks:
            new_insts = []
            for inst in blk.instructions:
                si = getattr(inst, "sync_info", None)
                ow = list(si.on_wait) if (si is not None and si.on_wait) else []
                if len(ow) > 1:
                    for w in ow[:-1]:
                        new_insts.append(_nop_like(inst, w))
                    inst.sync_info = mybir.SyncInfo(
                        on_wait=[ow[-1]], on_update=list(si.on_update or []))
                new_insts.append(inst)
            blk.instructions = new_insts


def _patched_drain_and_barrier(self, tick_clock, wait_clock):
    nc = self.nc
    drain_inst = nc.sync.drain()
    wait_clock.add_sem_waits(
        drain_inst.ins, ScopedClock({None: tick_clock.global_clock}))
    si = drain_inst.ins.sync_info
    if si is not None and si.on_wait is not None and len(si.on_wait) > 1:
        waits = list(si.on_wait)
        drain_inst.ins.sync_info = mybir.SyncInfo(
            on_wait=waits[:1], on_update=list(si.on_update or []))
        for w in waits[1:]:
            nop = nc.sync.nop()
            nop.ins.sync_info = mybir.SyncInfo(on_wait=[w], on_update=[])
    nc.all_engine_barrier()
    assert self.sems is not None
    popped = nc._tile_sem_poison_stack.pop()
    assert popped is self._sem_poison
    nc.clear_and_free_semaphores(list(self.sems.allocated().values()))
    nc.all_engine_barrier()


tile.TileContext._drain_and_barrier = _patched_drain_and_barrier

# ---------------------------------------------------------------------------


def _tf32_rne(a):
    """Round fp32 array to tf32 (13 low mantissa bits cleared), RNE."""
    u = np.ascontiguousarray(a, dtype=np.float32).view(np.uint32)
    lsb = (u >> np.uint32(13)) & np.uint32(1)
    r = (u + np.uint32(0xFFF) + lsb) & np.uint32(0xFFFFE000)
    return r.view(np.float32)


def _build_program():
    nc = bass.Bass(trn_type="TRN2", target_bir_lowering=False, debug=False,
                   num_devices=NCORES)

    xt = nc.dram_tensor("xt", [BL, D, T], F32R, kind="ExternalInput").ap()
    wts = [nc.dram_tensor(f"wt{w}", [D, D], F32R, kind="ExternalInput").ap()
           for w in range(3)]  # W^T for x, delta, gate: [d_in, d_out]
    bias = nc.dram_tensor("bias", [3, D, 1], F32, kind="ExternalInput").ap()
    rh_d = nc.dram_tensor("rh", [D, 1], F32, kind="ExternalInput").ap()
    rd_d = nc.dram_tensor("rd", [D, 1], F32, kind="ExternalInput").ap()
    omrh_d = nc.dram_tensor("omrh", [D, 1], F32, kind="ExternalInput").ap()  # 1 - r_h
    h0_d = nc.dram_tensor("h0", [BL, D, 1], F32, kind="ExternalInput").ap()

    out_d = nc.dram_tensor("out_dev", [BL, D, T], F32, kind="ExternalOutput").ap()
    h_out = nc.dram_tensor("h_dev", [BL, D, T + 1], F32, kind="ExternalOutput").ap()

    with tile.TileContext(nc) as tc:
        with tc.tile_pool(name="xb", bufs=1) as xbp, \
             tc.tile_pool(name="wp", bufs=1) as wp, \
             tc.tile_pool(name="cst", bufs=1) as cst, \
             tc.tile_pool(name="psum", bufs=2, space="PSUM") as pp, \
             tc.tile_pool(name="work", bufs=1) as wk, \
             tc.tile_pool(name="g", bufs=1) as gp:

            # constants: [128, EC] views of the [D,1] vectors
            rh_t = cst.tile([128, EC], F32, name="rh", tag="rh")
            rd_t = cst.tile([128, EC], F32, name="rd", tag="rd")
            omrh_t = cst.tile([128, EC], F32, name="omrh", tag="omrh")
            bias_t = [cst.tile([128, EC], F32, name=f"bias{w}", tag=f"bias{w}") for w in range(3)]
            for e in range(EC):
                sl = slice(e * 128, (e + 1) * 128)
                nc.sync.dma_start(rh_t[:, e:e + 1], rh_d[sl, :])
                nc.sync.dma_start(rd_t[:, e:e + 1], rd_d[sl, :])
                nc.sync.dma_start(omrh_t[:, e:e + 1], omrh_d[sl, :])
                for w in range(3):
                    nc.sync.dma_start(bias_t[w][:, e:e + 1], bias[w, sl, :])

            for b in range(BL):
                # x^T for this batch: 8 chunks of [128, T], f32r
                xb = []
                for k in range(KC):
                    t = xbp.tile([128, T], F32R, name=f"xb{k}", tag=f"xb{k}")
                    nc.sync.dma_start(t[:], xt[b, k * 128:(k + 1) * 128, :])
                    xb.append(t)

                for e in range(EC):
                    esl = slice(e * 128, (e + 1) * 128)
                    rh_s = rh_t[:, e:e + 1]
                    rd_s = rd_t[:, e:e + 1]
                    omrh_s = omrh_t[:, e:e + 1]

                    # ---- weights for this e-chunk
                    wt_e = []
                    for w in range(3):
                        wcol = []
                        for k in range(KC):
                            wt = wp.tile([128, 128], F32R, name=f"w{w}_{k}", tag=f"w{w}_{k}")
                            nc.sync.dma_start(
                                wt[:], wts[w][k * 128:(k + 1) * 128, esl])
                            wcol.append(wt)
                        wt_e.append(wcol)

                    # ---- GEMMs + evictions
                    XV = wk.tile([128, T], F32, name="XV", tag="XV")
                    XD = wk.tile([128, T], F32, name="XD", tag="XD")
                    XG = wk.tile([128, T], F32, name="XG", tag="XG")
                    SG = wk.tile([128, T], F32, name="SG", tag="SG")
                    U0 = wk.tile([128, T], F16, name="U0", tag="U0")
                    S0 = wk.tile([128, T], F16, name="S0", tag="S0")
                    for t4 in range(TC):
                        tsl = slice(t4 * 512, (t4 + 1) * 512)
                        pv = pp.tile([128, 512], F32, name="pv", tag="pv")
                        for k in range(KC):
                            nc.tensor.matmul(pv[:], wt_e[0][k][:], xb[k][:, tsl],
                                             start=(k == 0), stop=(k == KC - 1))
                        nc.scalar.activation(XV[:, tsl], pv[:], AF.Identity,
                                             bias=bias_t[0][:, e:e + 1])
                        nc.scalar.activation(U0[:, tsl], pv[:], AF.Tanh,
                                             bias=bias_t[0][:, e:e + 1])

                        pd = pp.tile([128, 512], F32, name="pd", tag="pd")
                        for k in range(KC):
                            nc.tensor.matmul(pd[:], wt_e[1][k][:], xb[k][:, tsl],
                                             start=(k == 0), stop=(k == KC - 1))
                        nc.scalar.activation(XD[:, tsl], pd[:], AF.Identity,
                                             bias=bias_t[1][:, e:e + 1])
                        nc.scalar.activation(S0[:, tsl], pd[:], AF.Sigmoid,
                                             bias=bias_t[1][:, e:e + 1])

                        pg = pp.tile([128, 512], F32, name="pg", tag="pg")
                        for k in range(KC):
                            nc.tensor.matmul(pg[:], wt_e[2][k][:], xb[k][:, tsl],
                                             start=(k == 0), stop=(k == KC - 1))
                        nc.scalar.activation(XG[:, tsl], pg[:], AF.Identity,
                                             bias=bias_t[2][:, e:e + 1])
                        nc.scalar.activation(SG[:, tsl], pg[:], AF.Sigmoid,
                                             bias=bias_t[2][:, e:e + 1])

                    # ---- state tiles
                    g32 = gp.tile([128, T + 1], F32, name="g32", tag="g32")
                    g16 = wk.tile([128, T + 1], F16, name="g16", tag="g16")
                    nc.sync.dma_start(g32[:, 0:1], h0_d[b, esl, :])
                    nc.scalar.activation(g16[:, 0:1], g32[:, 0:1], AF.Copy)

                    # ---- init scan: a0 = 1-S0, b0 = S0*U0
                    a0 = wk.tile([128, T], F16, name="tV16", tag="tV16")
                    nc.vector.tensor_scalar(a0[:], S0[:], -1.0, 1.0,
                                            ALU.mult, ALU.add)
                    b0 = wk.tile([128, T], F16, name="tD16", tag="tD16")
                    nc.gpsimd.tensor_mul(b0[:], S0[:], U0[:])
                    nc.vector.tensor_tensor_scan(
                        g16[:, 1:], a0[:], b0[:], g16[:, 0:1], ALU.mult, ALU.add)

                    # ---- Newton iterations
                    for it in range(NITER):
                        last = (it == NITER - 1)
                        dt = F32 if last else F16
                        tg = "32" if last else "16"
                        tV = wk.tile([128, T], dt, name=f"tV{tg}", tag=f"tV{tg}")
                        tD = wk.tile([128, T], dt, name=f"tD{tg}", tag=f"tD{tg}")
                        tU = wk.tile([128, T], dt, name=f"tU{tg}", tag=f"tU{tg}")
                        tS = wk.tile([128, T], dt, name=f"tS{tg}", tag=f"tS{tg}")
                        tU2 = wk.tile([128, T], dt, name=f"tU2{tg}", tag=f"tU2{tg}")

                        gprev = g16[:, 0:T]
                        # V = r_h*g + XV ; D = r_delta*g + XD
                        nc.vector.scalar_tensor_tensor(
                            tV[:], gprev, rh_s, XV[:], ALU.mult, ALU.add)
                        nc.vector.scalar_tensor_tensor(
                            tD[:], gprev, rd_s, XD[:], ALU.mult, ALU.add)
                        nc.scalar.activation(tU[:], tV[:], AF.Tanh)
                        nc.scalar.activation(tS[:], tD[:], AF.Sigmoid)
                        nc.scalar.activation(tU2[:], tU[:], AF.Square)
                        # s = U - g (into tV) ; p = S*s (into tD)
                        nc.vector.tensor_sub(tV[:], tU[:], gprev)
                        nc.vector.tensor_mul(tD[:], tS[:], tV[:])
                        # k = r_h*U2 + (1-r_h)  (into tU)
                        nc.vector.tensor_scalar(tU[:], tU2[:], rh_s, omrh_s,
                                                ALU.mult, ALU.add)
                        # A' = S*k (into tU2)
                        nc.vector.tensor_mul(tU2[:], tS[:], tU[:])
                        # m = A'*g (into tS) ; Bs = m + p (into tV)
                        nc.gpsimd.tensor_mul(tS[:], tU2[:], gprev)
                        nc.gpsimd.tensor_add(tV[:], tS[:], tD[:])
                        # a = 1 - A' (into tD)
                        nc.vector.tensor_scalar(tD[:], tU2[:], -1.0, 1.0,
                                                ALU.mult, ALU.add)
                        gout = g32 if last else g16
                        nc.vector.tensor_tensor_scan(
                            gout[:, 1:], tD[:], tV[:], gout[:, 0:1],
                            ALU.mult, ALU.add)

                    # ---- outputs: out = h * xg * sigmoid(xg)
                    o1 = wk.tile([128, T], F32, name="tU32", tag="tU32")
                    nc.gpsimd.tensor_mul(o1[:], g32[:, 1:], XG[:])
                    o2 = wk.tile([128, T], F32, name="tS32", tag="tS32")
                    nc.gpsimd.tensor_mul(o2[:], o1[:], SG[:])
                    nc.sync.dma_start(out_d[b, esl, :], o2[:])
                    nc.sync.dma_start(h_out[b, esl, :], g32[:])

    _split_multi_waits(nc)
    return nc


_prog_cache = {}


def _get_program():
    if "nc" not in _prog_cache:
        _prog_cache["nc"] = _build_program()
    return _prog_cache["nc"]


def kernel(x, h0, A_log, r_delta, W_x, W_delta, W_gate, b, b_delta, b_gate,
           _profile=False):
    x = np.asarray(x, dtype=np.float32)
    h0 = np.asarray(h0, dtype=np.float32)
    A_log = np.asarray(A_log, dtype=np.float32)
    r_delta = np.asarray(r_delta, dtype=np.float32)

    nc = _get_program()

    r_h = (-np.exp(A_log)).astype(np.float32)
    rh_v = r_h.reshape(D, 1)
    rd_v = r_delta.reshape(D, 1).astype(np.float32)
    omrh_v = (1.0 - r_h).reshape(D, 1).astype(np.float32)
    bias_v = np.stack([np.asarray(v, dtype=np.float32).reshape(D, 1)
                       for v in (b, b_delta, b_gate)])  # [3, D, 1]

    wt_arrs = [_tf32_rne(np.ascontiguousarray(np.asarray(w, np.float32).T))
               for w in (W_x, W_delta, W_gate)]
    xT = _tf32_rne(np.ascontiguousarray(x.transpose(1, 2, 0)))  # [B, D, T]
    h0r = h0.reshape(B, D, 1)

    in_maps = []
    for c in range(NCORES):
        bs = slice(c * BL, (c + 1) * BL)
        m = {"xt": np.ascontiguousarray(xT[bs]),
             "bias": bias_v, "rh": rh_v, "rd": rd_v, "omrh": omrh_v,
             "h0": np.ascontiguousarray(h0r[bs])}
        for w in range(3):
            m[f"wt{w}"] = wt_arrs[w]
        in_maps.append(m)

    res = run_bass_kernel_spmd(nc, in_maps, core_ids=list(range(NCORES)),
                               trace=_profile)
    if _profile and res.exec_time_ns is not None:
        print(f"HW exec time: {res.exec_time_ns} ns")

    out_all = np.stack([res.results[c]["out_dev"] for c in range(NCORES)])
    h_all = np.stack([res.results[c]["h_dev"] for c in range(NCORES)])
    # [core, bl, d, t] -> [t, core*BL+bl, d]
    output = np.ascontiguousarray(
        out_all.transpose(3, 0, 1, 2).reshape(T, B, D))
    h = np.ascontiguousarray(
        h_all.transpose(3, 0, 1, 2).reshape(T + 1, B, D))
    return output, h


# revision 6
# speedup vs baseline: 1.0765x; 1.0765x over previous
"""Trainium2 Bass kernel for nn_DiagTripleRCell.

Math (per (b, d) element, T steps):
    xv = W_x x + b ; xd = W_delta x + b_delta ; xg = W_gate x + b_gate
    h_{t+1} = (1-delta_t) h_t + delta_t tanh(xv_t + r_h h_t),
      delta_t = sigmoid(xd_t + r_delta h_t)
    out_t = h_{t+1} * silu(xg_t)

Strategy: batch-parallel over 8 cores (B=16 -> 2 per core). GEMMs run on
the PE in f32r (tf32) with outputs laid [d_out partitions, time free] so
the recurrence can use the DVE hardware affine scan (tensor_tensor_scan).
The nonlinear recurrence is solved parallel-in-time by Newton iteration:
an h-independent affine-scan initial guess, then NITER Newton passes,
each of which linearizes around the current trajectory and solves the
resulting affine recurrence exactly with one scan per (d-chunk, b) pair.
Iteration 1 runs in fp16 (accuracy only needs to reach ~3e-3 there),
the final iteration in fp32.
"""
import sys

sys.path.insert(0, "/opt/trn_rl_repo")

import numpy as np

import concourse.bass as bass
import concourse.tile as tile
from concourse import mybir
from concourse.bass_utils import run_bass_kernel_spmd
from concourse.vector_clock import ScopedClock

F32 = mybir.dt.float32
F32R = mybir.dt.float32r
F16 = mybir.dt.float16
AF = mybir.ActivationFunctionType
ALU = mybir.AluOpType

T, B, D = 2048, 16, 1024
NCORES = 8
BL = B // NCORES          # batches per core
EC = D // 128             # output-d chunks
KC = D // 128             # contraction chunks
TC = T // 512             # psum column chunks
NITER = 2                 # Newton iterations after the init scan

# ---------------------------------------------------------------------------
# walrus workaround: this container's walrus accepts at most ONE sem wait per
# instruction; split extras onto single-wait NOPs.
_uid = [0]


def _nop_like(inst, wait):
    _uid[0] += 1
    return mybir.InstNoOp(
        name=f"waitnop_{_uid[0]}", ins=[], outs=[], engine=inst.engine,
        sync_info=mybir.SyncInfo(on_wait=[wait], on_update=[]),
    )


def _split_multi_waits(nc):
    for fn in nc.m.functions:
        for blk in fn.blocks:
            new_insts = []
            for inst in blk.instructions:
                si = getattr(inst, "sync_info", None)
                ow = list(si.on_wait) if (si is not None and si.on_wait) else []
                if len(ow) > 1:
                    for w in ow[:-1]:
                        new_insts.append(_nop_like(inst, w))
                    inst.sync_info = mybir.SyncInfo(
                        on_wait=[ow[-1]], on_update=list(si.on_update or []))
                new_insts.append(inst)
            blk.instructions = new_insts


def _patched_drain_and_barrier(self, tick_clock, wait_clock):
    nc = self.nc
    drain_inst = nc.sync.drain()
    wait_clock.add_sem_waits(
        drain_inst.ins, ScopedClock({None: tick_clock.global_clock}))
    si = drain_inst.ins.sync_info
    if si is not None and si.on_wait is not None and len(si.on_wait) > 1:
        waits = list(si.on_wait)
        drain_inst.ins.sync_info = mybir.SyncInfo(
            on_wait=waits[:1], on_update=list(si.on_update or []))
        for w in waits[1:]:
            nop = nc.sync.nop()
            nop.ins.sync_info = mybir.SyncInfo(on_wait=[w], on_update=[])
    nc.all_engine_barrier()
    assert self.sems is not None
    popped = nc._tile_sem_poison_stack.pop()
    assert popped is self._sem_poison
    nc.clear_and_free_semaphores(list(self.sems.allocated().values()))
    nc.all_engine_barrier()


tile.TileContext._drain_and_barrier = _patched_drain_and_barrier

# ---------------------------------------------------------------------------


def _tf32_rne(a):
    """Round fp32 array to tf32 (13 low mantissa bits cleared), RNE."""
    u = np.ascontiguousarray(a, dtype=np.float32).view(np.uint32)
    lsb = (u >> np.uint32(13)) & np.uint32(1)
    r = (u + np.uint32(0xFFF) + lsb) & np.uint32(0xFFFFE000)
    return r.view(np.float32)


def _build_program():
    nc = bass.Bass(trn_type="TRN2", target_bir_lowering=False, debug=False,
                   num_devices=NCORES)

    xt = nc.dram_tensor("xt", [BL, D, T], F32R, kind="ExternalInput").ap()
    wts = [nc.dram_tensor(f"wt{w}", [D, D], F32R, kind="ExternalInput").ap()
           for w in range(3)]  # W^T for x, delta, gate: [d_in, d_out]
    bias = nc.dram_tensor("bias", [3, D, 1], F32, kind="ExternalInput").ap()
    rh_d = nc.dram_tensor("rh", [D, 1], F32, kind="ExternalInput").ap()
    rd_d = nc.dram_tensor("rd", [D, 1], F32, kind="ExternalInput").ap()
    omrh_d = nc.dram_tensor("omrh", [D, 1], F32, kind="ExternalInput").ap()  # 1 - r_h
    h0_d = nc.dram_tensor("h0", [BL, D, 1], F32, kind="ExternalInput").ap()

    out_d = nc.dram_tensor("out_dev", [BL, D, T], F32, kind="ExternalOutput").ap()
    h_out = nc.dram_tensor("h_dev", [BL, D, T + 1], F32, kind="ExternalOutput").ap()

    with tile.TileContext(nc) as tc:
        with tc.tile_pool(name="xb", bufs=1) as xbp, \
             tc.tile_pool(name="wp", bufs=1) as wp, \
             tc.tile_pool(name="cst", bufs=1) as cst, \
             tc.tile_pool(name="psum", bufs=2, space="PSUM") as pp, \
             tc.tile_pool(name="work", bufs=2) as wk:

            # constants: [128, EC] views of the [D,1] vectors
            rh_t = cst.tile([128, EC], F32, name="rh", tag="rh")
            rd_t = cst.tile([128, EC], F32, name="rd", tag="rd")
            omrh_t = cst.tile([128, EC], F32, name="omrh", tag="omrh")
            bias_t = [cst.tile([128, EC], F32, name=f"bias{w}", tag=f"bias{w}")
                      for w in range(3)]
            for e in range(EC):
                sl = slice(e * 128, (e + 1) * 128)
                nc.sync.dma_start(rh_t[:, e:e + 1], rh_d[sl, :])
                nc.sync.dma_start(rd_t[:, e:e + 1], rd_d[sl, :])
                nc.sync.dma_start(omrh_t[:, e:e + 1], omrh_d[sl, :])
                for w in range(3):
                    nc.sync.dma_start(bias_t[w][:, e:e + 1], bias[w, sl, :])

            for b in range(BL):
                # x^T for this batch: 8 chunks of [128, T], f32r
                xb = []
                for k in range(KC):
                    t = xbp.tile([128, T], F32R, name=f"xb{k}", tag=f"xb{k}")
                    nc.sync.dma_start(t[:], xt[b, k * 128:(k + 1) * 128, :])
                    xb.append(t)

                for e in range(EC):
                    esl = slice(e * 128, (e + 1) * 128)
                    rh_s = rh_t[:, e:e + 1]
                    rd_s = rd_t[:, e:e + 1]
                    omrh_s = omrh_t[:, e:e + 1]

                    # ---- weights for this e-chunk
                    wt_e = []
                    for w in range(3):
                        wcol = []
                        for k in range(KC):
                            wt = wp.tile([128, 128], F32R,
                                         name=f"w{w}_{k}", tag=f"w{w}_{k}")
                            nc.sync.dma_start(
                                wt[:], wts[w][k * 128:(k + 1) * 128, esl])
                            wcol.append(wt)
                        wt_e.append(wcol)

                    # ---- per-pair tiles (4 shared temp slots, 8KB each)
                    XV = wk.tile([128, T], F32, name="XV", tag="XV")
                    XD = wk.tile([128, T], F32, name="XD", tag="XD")
                    XG = wk.tile([128, T], F32, name="XG", tag="XG")
                    SG = wk.tile([128, T], F32, name="SG", tag="tD")
                    U0 = wk.tile([128, T], F16, name="U0", tag="tU")
                    S0 = wk.tile([128, T], F16, name="S0", tag="tS")
                    g16 = wk.tile([128, T + 1], F16, name="g16", tag="g16")
                    g32 = wk.tile([128, T + 1], F32, name="g32", tag="g32", bufs=1)

                    # ---- GEMMs + evictions
                    for t4 in range(TC):
                        tsl = slice(t4 * 512, (t4 + 1) * 512)
                        pv = pp.tile([128, 512], F32, name="pv", tag="pv")
                        for k in range(KC):
                            nc.tensor.matmul(pv[:], wt_e[0][k][:], xb[k][:, tsl],
                                             start=(k == 0), stop=(k == KC - 1))
                        nc.scalar.activation(XV[:, tsl], pv[:], AF.Identity,
                                             bias=bias_t[0][:, e:e + 1])
                        nc.scalar.activation(U0[:, tsl], pv[:], AF.Tanh,
                                             bias=bias_t[0][:, e:e + 1])

                        pd = pp.tile([128, 512], F32, name="pd", tag="pd")
                        for k in range(KC):
                            nc.tensor.matmul(pd[:], wt_e[1][k][:], xb[k][:, tsl],
                                             start=(k == 0), stop=(k == KC - 1))
                        nc.scalar.activation(XD[:, tsl], pd[:], AF.Identity,
                                             bias=bias_t[1][:, e:e + 1])
                        nc.scalar.activation(S0[:, tsl], pd[:], AF.Sigmoid,
                                             bias=bias_t[1][:, e:e + 1])

                        pg = pp.tile([128, 512], F32, name="pg", tag="pg")
                        for k in range(KC):
                            nc.tensor.matmul(pg[:], wt_e[2][k][:], xb[k][:, tsl],
                                             start=(k == 0), stop=(k == KC - 1))
                        nc.scalar.activation(XG[:, tsl], pg[:], AF.Identity,
                                             bias=bias_t[2][:, e:e + 1])
                        nc.scalar.activation(SG[:, tsl], pg[:], AF.Sigmoid,
                                             bias=bias_t[2][:, e:e + 1])

                    # gate = xg * sigmoid(xg), in place over XG; frees SG slot
                    nc.gpsimd.tensor_mul(XG[:], XG[:], SG[:])

                    # ---- state init
                    nc.sync.dma_start(g32[:, 0:1], h0_d[b, esl, :])
                    nc.scalar.activation(g16[:, 0:1], g32[:, 0:1], AF.Copy)

                    # ---- init scan: a0 = 1-S0, b0 = S0*U0
                    a0 = wk.tile([128, T], F16, name="a0", tag="tD")
                    nc.vector.tensor_scalar(a0[:], S0[:], -1.0, 1.0,
                                            ALU.mult, ALU.add)
                    b0 = wk.tile([128, T], F16, name="b0", tag="tV")
                    nc.gpsimd.tensor_mul(b0[:], S0[:], U0[:])
                    nc.vector.tensor_tensor_scan(
                        g16[:, 1:], a0[:], b0[:], g16[:, 0:1], ALU.mult, ALU.add)

                    # ---- Newton iterations (4 temp slots: tV tD tU tS)
                    for it in range(NITER):
                        last = (it == NITER - 1)
                        dt = F32 if last else F16
                        tV = wk.tile([128, T], dt, name="tV", tag="tV")
                        tD = wk.tile([128, T], dt, name="tD", tag="tD")
                        gprev = g16[:, 0:T]
                        # V = r_h*g + XV ; D = r_delta*g + XD
                        nc.vector.scalar_tensor_tensor(
                            tV[:], gprev, rh_s, XV[:], ALU.mult, ALU.add)
                        nc.vector.scalar_tensor_tensor(
                            tD[:], gprev, rd_s, XD[:], ALU.mult, ALU.add)
                        tU = wk.tile([128, T], dt, name="tU", tag="tU")
                        nc.scalar.activation(tU[:], tV[:], AF.Tanh)       # U
                        tS = wk.tile([128, T], dt, name="tS", tag="tS")
                        nc.scalar.activation(tS[:], tD[:], AF.Sigmoid)    # S
                        tU2 = wk.tile([128, T], dt, name="tU2", tag="tV")
                        nc.scalar.activation(tU2[:], tU[:], AF.Square)    # U2 (slot V)
                        ts_ = wk.tile([128, T], dt, name="ts_", tag="tD")
                        nc.vector.tensor_sub(ts_[:], tU[:], gprev)        # s (slot D)
                        tp = wk.tile([128, T], dt, name="tp", tag="tU")
                        nc.vector.tensor_mul(tp[:], tS[:], ts_[:])        # p (slot U)
                        tk = wk.tile([128, T], dt, name="tk", tag="tD")
                        nc.vector.tensor_scalar(tk[:], tU2[:], rh_s, omrh_s,
                                                ALU.mult, ALU.add)        # k (slot D)
                        tA = wk.tile([128, T], dt, name="tA", tag="tV")
                        nc.vector.tensor_mul(tA[:], tS[:], tk[:])         # A' (slot V)
                        tm = wk.tile([128, T], dt, name="tm", tag="tD")
                        nc.gpsimd.tensor_mul(tm[:], tA[:], gprev)         # m (slot D)
                        nc.gpsimd.tensor_add(tp[:], tm[:], tp[:])         # B (over p)
                        ta = wk.tile([128, T], dt, name="ta", tag="tS")
                        nc.vector.tensor_scalar(ta[:], tA[:], -1.0, 1.0,
                                                ALU.mult, ALU.add)        # a (slot S)
                        gout = g32 if last else g16
                        nc.vector.tensor_tensor_scan(
                            gout[:, 1:], ta[:], tp[:], gout[:, 0:1],
                            ALU.mult, ALU.add)

                    # ---- outputs: out = h * gate
                    o1 = wk.tile([128, T], F32, name="o1", tag="tV")
                    nc.gpsimd.tensor_mul(o1[:], g32[:, 1:], XG[:])
                    nc.sync.dma_start(out_d[b, esl, :], o1[:])
                    nc.sync.dma_start(h_out[b, esl, :], g32[:])

    _split_multi_waits(nc)
    return nc


_prog_cache = {}


def _get_program():
    if "nc" not in _prog_cache:
        _prog_cache["nc"] = _build_program()
    return _prog_cache["nc"]


def kernel(x, h0, A_log, r_delta, W_x, W_delta, W_gate, b, b_delta, b_gate,
           _profile=False):
    x = np.asarray(x, dtype=np.float32)
    h0 = np.asarray(h0, dtype=np.float32)
    A_log = np.asarray(A_log, dtype=np.float32)
    r_delta = np.asarray(r_delta, dtype=np.float32)

    nc = _get_program()

    r_h = (-np.exp(A_log)).astype(np.float32)
    rh_v = r_h.reshape(D, 1)
    rd_v = r_delta.reshape(D, 1).astype(np.float32)
    omrh_v = (1.0 - r_h).reshape(D, 1).astype(np.float32)
    bias_v = np.stack([np.asarray(v, dtype=np.float32).reshape(D, 1)
                       for v in (b, b_delta, b_gate)])  # [3, D, 1]

    wt_arrs = [_tf32_rne(np.ascontiguousarray(np.asarray(w, np.float32).T))
               for w in (W_x, W_delta, W_gate)]
    xT = _tf32_rne(np.ascontiguousarray(x.transpose(1, 2, 0)))  # [B, D, T]
    h0r = h0.reshape(B, D, 1)

    in_maps = []
    for c in range(NCORES):
        bs = slice(c * BL, (c + 1) * BL)
        m = {"xt": np.ascontiguousarray(xT[bs]),
             "bias": bias_v, "rh": rh_v, "rd": rd_v, "omrh": omrh_v,
             "h0": np.ascontiguousarray(h0r[bs])}
        for w in range(3):
            m[f"wt{w}"] = wt_arrs[w]
        in_maps.append(m)

    res = run_bass_kernel_spmd(nc, in_maps, core_ids=list(range(NCORES)),
                               trace=_profile)
    if _profile and res.exec_time_ns is not None:
        print(f"HW exec time: {res.exec_time_ns} ns")

    out_all = np.stack([res.results[c]["out_dev"] for c in range(NCORES)])
    h_all = np.stack([res.results[c]["h_dev"] for c in range(NCORES)])
    # [core, bl, d, t] -> [t, core*BL+bl, d]
    output = np.ascontiguousarray(
        out_all.transpose(3, 0, 1, 2).reshape(T, B, D))
    h = np.ascontiguousarray(
        h_all.transpose(3, 0, 1, 2).reshape(T + 1, B, D))
    return output, h
